# revision 3
# baseline (speedup 1.0000x reference)
import os
import sys
import threading
import time

import numpy as np

sys.path.insert(0, "/opt/trn_rl_repo")

from contextlib import ExitStack

import ml_dtypes

from concourse import bass, mybir, tile
from concourse.masks import make_identity

F32 = mybir.dt.float32
BF16 = mybir.dt.bfloat16
U32 = mybir.dt.uint32
AF = mybir.ActivationFunctionType
BF16_NP = ml_dtypes.bfloat16

B, N, S = 8, 8192, 2048
D1, D2, Cin, C1, C2 = 128, 256, 384, 256, 128
P = 128
NT = N // P  # 64 column tiles of 128
NG = NT // 4  # 16 groups of 4 tiles (512 cols)
TOT = float(B * N)
NN_EPS = 1e-8
BN_EPS = 1e-5
N_CORES = 8

last = {}


def _build_nc():
    nc = bass.Bass()

    # Batched inputs (fewer tunnel transfers):
    # dlcat rows 0:15 = distance lhsT for tiles t%3==0/1/2 at column block
    # t//3 (bands zero-padded to 2816); rows 15:20 cols 0:2048 = [x2; -1; n2].
    # wcat rows 0:384 = w1T (Cin,C1); rows 384:640 cols 0:128 = w2T (C1,C2).
    # bncat cols: b1r 0:2 | g1r 2:4 | be1r 4:6 | b2r 6 | g2r 7 | be2r 8.
    dlcat = nc.declare_dram_parameter("dlcat", [20, 2816], F32, isOutput=False)
    p2t = nc.declare_dram_parameter("p2t", [S, D2], BF16, isOutput=False)
    points1 = nc.declare_dram_parameter("points1", [D1, N], BF16, isOutput=False)
    wcat = nc.declare_dram_parameter("wcat", [Cin + C1, C1], BF16, isOutput=False)
    bncat = nc.declare_dram_parameter("bncat", [P, 9], F32, isOutput=False)
    out = nc.declare_dram_parameter("out", [C2, N], BF16, isOutput=True)

    with tile.TileContext(nc) as tc, ExitStack() as ctx:
        consts = ctx.enter_context(tc.tile_pool(name="consts", bufs=1))
        # p1d: DMA-landing tiles; bufs=8 matches the 8 HW-DGE queues so
        # buffer-reuse WAW lands on the same queue semaphore as the own-queue
        # wait (DMA structs allow 2 waits).
        p1d_pool = ctx.enter_context(tc.tile_pool(name="p1d", bufs=8))
        score_pool = ctx.enter_context(tc.tile_pool(name="score", bufs=2))
        topk_pool = ctx.enter_context(tc.tile_pool(name="topk", bufs=6))
        wt_pool = ctx.enter_context(tc.tile_pool(name="wt", bufs=8))
        gath_pool = ctx.enter_context(tc.tile_pool(name="gath", bufs=8))
        interp_pool = ctx.enter_context(tc.tile_pool(name="interp", bufs=3))
        xT_pool = ctx.enter_context(tc.tile_pool(name="xT", bufs=3))
        scratch_pool = ctx.enter_context(tc.tile_pool(name="scratch", bufs=2))
        outc_pool = ctx.enter_context(tc.tile_pool(name="outc", bufs=2))
        psum_d = ctx.enter_context(tc.tile_pool(name="psum_d", bufs=2, space="PSUM"))
        psum_t = ctx.enter_context(tc.tile_pool(name="psum_t", bufs=2, space="PSUM"))
        psum_c = ctx.enter_context(tc.tile_pool(name="psum_c", bufs=2, space="PSUM"))
        dram = ctx.enter_context(tc.tile_pool(name="dram", bufs=4, space="DRAM"))

        # ---- constants ----
        # Matmul operands are staged through a Pool-engine copy so PE waits
        # collapse onto one compute semaphore (HW-DGE queue fan-out otherwise
        # exceeds the Matmult struct's sync-wait slots). Copies are band-wise
        # so no uninitialized SBUF is touched.
        rhs_sb = consts.tile((69, S), F32)
        dl_sb = consts.tile((69, 2816), F32)
        for r in range(3):
            base = 32 * r
            nc.sync.dma_start(rhs_sb[base : base + 5, :], dlcat[15:20, 0:S])
            nc.gpsimd.tensor_copy(rhs_sb[base : base + 5, :], rhs_sb[base : base + 5, :])
            nc.sync.dma_start(dl_sb[base : base + 5, :], dlcat[5 * r : 5 * r + 5, :])
            nc.gpsimd.tensor_copy(dl_sb[base : base + 5, :], dl_sb[base : base + 5, :])
        w1c = []
        for kc in range(3):
            wt = consts.tile((P, C1), BF16, name=f"w1c{kc}")
            nc.sync.dma_start(wt[:], wcat[kc * P : (kc + 1) * P, :])
            nc.gpsimd.tensor_copy(wt[:], wt[:])
            w1c.append(wt)
        w2c = []
        for kc in range(2):
            wt = consts.tile((P, C2), BF16, name=f"w2c{kc}")
            nc.sync.dma_start(wt[:], wcat[Cin + kc * P : Cin + (kc + 1) * P, 0:C2])
            nc.gpsimd.tensor_copy(wt[:], wt[:])
            w2c.append(wt)
        bn_sb = consts.tile((P, 9), F32)
        nc.sync.dma_start(bn_sb[:], bncat[:])
        b1_sb = bn_sb[:, 0:2]
        g1_sb = bn_sb[:, 2:4]
        be1_sb = bn_sb[:, 4:6]
        b2_sb = bn_sb[:, 6:7]
        g2_sb = bn_sb[:, 7:8]
        be2_sb = bn_sb[:, 8:9]
        ident = consts.tile((P, P), F32)
        make_identity(nc, ident[:])
        eps_sb = consts.tile((P, 1), F32)
        nc.vector.memset(eps_sb[:], BN_EPS)

        # ---- persistent activations / stats ----
        y1h = [consts.tile((P, N), F32, name=f"y1h{o}") for o in range(2)]
        y2 = consts.tile((P, N), F32)
        sums1 = [consts.tile((P, NG), F32, name=f"sums1_{o}") for o in range(2)]
        sq1 = [consts.tile((P, NG), F32, name=f"sq1_{o}") for o in range(2)]
        sums2 = consts.tile((P, NG), F32)
        sq2 = consts.tile((P, NG), F32)

        # ---- Phase A: distances, top-3, gather, interp, conv1 ----
        # Software pipeline at group granularity (4 tiles = 512 cols):
        # stage1(g) issues DMAs/matmuls/top-k/gathers for 4 tiles, then the
        # previous group's interp+transpose+conv1 consumes them.
        stage = [None] * NT

        def stage1(t):
            jb = t // 3
            base = 32 * (t % 3)
            lt = dl_sb[base : base + 5, jb * P : (jb + 1) * P]

            score = score_pool.tile((P, S), F32)
            for c in range(2):
                ps = psum_d.tile((P, 1024), F32)
                for h in range(2):
                    nc.tensor.matmul(
                        ps[:, h * 512 : (h + 1) * 512],
                        lhsT=lt,
                        rhs=rhs_sb[
                            base : base + 5,
                            c * 1024 + h * 512 : c * 1024 + (h + 1) * 512,
                        ],
                        start=True,
                        stop=True,
                    )
                nc.scalar.copy(score[:, c * 1024 : (c + 1) * 1024], ps[:])

            maxv = topk_pool.tile((P, 8), F32)
            nc.vector.max(maxv[:], score[:])
            idx = topk_pool.tile((P, 8), U32)
            nc.vector.max_index(idx[:], maxv[:], score[:])

            # dist_k = -score_k ; recip = 1/(dist+eps); wn = recip/sum(recip)
            dist3 = wt_pool.tile((P, 3), F32)
            nc.scalar.activation(dist3[:], maxv[:, 0:3], AF.Copy, bias=NN_EPS, scale=-1.0)
            recipv = wt_pool.tile((P, 3), F32)
            nc.vector.reciprocal(recipv[:], dist3[:])
            rsum = wt_pool.tile((P, 1), F32)
            nc.vector.reduce_sum(rsum[:], recipv[:], axis=mybir.AxisListType.X)
            invs = wt_pool.tile((P, 1), F32)
            nc.vector.reciprocal(invs[:], rsum[:])
            wn = wt_pool.tile((P, 3), F32)
            nc.scalar.activation(wn[:], recipv[:], AF.Copy, scale=invs[:])

            gs = []
            for k in range(3):
                g = gath_pool.tile((P, D2), BF16, name=f"g{k}")
                nc.gpsimd.indirect_dma_start(
                    out=g[:],
                    out_offset=None,
                    in_=p2t[:],
                    in_offset=bass.IndirectOffsetOnAxis(ap=idx[:, k : k + 1], axis=0),
                )
                gs.append(g)
            return (wn, gs)

        def stage2a(t, tloc, it01):
            wn, gs = stage[t]
            acc = interp_pool.tile((P, D2), F32)
            nc.scalar.activation(acc[:], gs[0][:], AF.Copy, scale=wn[:, 0:1])
            tmp = interp_pool.tile((P, D2), F32)
            nc.scalar.activation(tmp[:], gs[1][:], AF.Copy, scale=wn[:, 1:2])
            nc.vector.tensor_add(acc[:], acc[:], tmp[:])
            nc.scalar.activation(tmp[:], gs[2][:], AF.Copy, scale=wn[:, 2:3])
            nc.vector.tensor_add(acc[:], acc[:], tmp[:])

            for h in range(2):
                tp = psum_t.tile((P, P), F32)
                nc.tensor.transpose(tp[:], acc[:, h * P : (h + 1) * P], ident[:])
                nc.scalar.copy(it01[h][:, tloc * P : (tloc + 1) * P], tp[:])
            stage[t] = None

        def do_group(g, p1d):
            it0 = xT_pool.tile((P, 512), BF16, name="it0")
            it1 = xT_pool.tile((P, 512), BF16, name="it1")
            for i in range(4):
                stage2a(4 * g + i, i, (it0, it1))
            p1f = xT_pool.tile((P, 512), BF16, name="p1f")
            nc.gpsimd.tensor_copy(p1f[:], p1d[:])
            rhs3 = [p1f, it0, it1]
            for o in range(2):
                yps = psum_c.tile((P, 512), F32)
                for kc in range(3):
                    nc.tensor.matmul(
                        yps[:],
                        lhsT=w1c[kc][:, o * P : (o + 1) * P],
                        rhs=rhs3[kc][:],
                        start=(kc == 0),
                        stop=(kc == 2),
                    )
                nc.scalar.activation(
                    y1h[o][:, g * 512 : (g + 1) * 512],
                    yps[:],
                    AF.Identity,
                    bias=b1_sb[:, o : o + 1],
                    accum_out=sums1[o][:, g : g + 1],
                )
                sc = scratch_pool.tile((P, 512), F32)
                nc.scalar.activation(
                    sc[:],
                    y1h[o][:, g * 512 : (g + 1) * 512],
                    AF.Square,
                    accum_out=sq1[o][:, g : g + 1],
                )

        p1ds = [None] * NG
        for g in range(NG + 1):
            if g < NG:
                p1d = p1d_pool.tile((P, 512), BF16)
                nc.sync.dma_start(p1d[:], points1[:, g * 512 : (g + 1) * 512])
                p1ds[g] = p1d
                for i in range(4):
                    stage[4 * g + i] = stage1(4 * g + i)
            if g >= 1:
                do_group(g - 1, p1ds[g - 1])
                p1ds[g - 1] = None

        # ---- BN1 stats AllReduce ----
        stats1 = consts.tile((P, 4), F32)
        nc.vector.reduce_sum(stats1[:, 0:1], sums1[0][:], axis=mybir.AxisListType.X)
        nc.vector.reduce_sum(stats1[:, 1:2], sums1[1][:], axis=mybir.AxisListType.X)
        nc.vector.reduce_sum(stats1[:, 2:3], sq1[0][:], axis=mybir.AxisListType.X)
        nc.vector.reduce_sum(stats1[:, 3:4], sq1[1][:], axis=mybir.AxisListType.X)
        st1_in = dram.tile((P, 4), F32)
        st1_out = dram.tile((P, 4), F32)
        nc.gpsimd.dma_start(st1_in[:], stats1[:])
        nc.gpsimd.collective_compute(
            "AllReduce",
            mybir.AluOpType.add,
            replica_groups=[list(range(N_CORES))],
            ins=[st1_in.opt()],
            outs=[st1_out.opt()],
        )
        ared1 = consts.tile((P, 4), F32)
        nc.gpsimd.dma_start(ared1[:], st1_out[:])

        # scale s = gamma/sqrt(var+eps), shift t = beta - mean*s
        def bn_params(ared, nch, g_sb, be_sb):
            m = consts.tile((P, nch), F32)
            nc.scalar.activation(m[:], ared[:, 0:nch], AF.Copy, scale=1.0 / TOT)
            ex2 = consts.tile((P, nch), F32)
            nc.scalar.activation(ex2[:], ared[:, nch : 2 * nch], AF.Copy, scale=1.0 / TOT)
            msq = consts.tile((P, nch), F32)
            nc.scalar.activation(msq[:], m[:], AF.Square)
            var = consts.tile((P, nch), F32)
            nc.vector.tensor_sub(var[:], ex2[:], msq[:])
            sd = consts.tile((P, nch), F32)
            nc.scalar.activation(sd[:], var[:], AF.Sqrt, bias=eps_sb[:])
            rs = consts.tile((P, nch), F32)
            nc.vector.reciprocal(rs[:], sd[:])
            s = consts.tile((P, nch), F32)
            nc.vector.tensor_mul(s[:], rs[:], g_sb[:])
            ms = consts.tile((P, nch), F32)
            nc.vector.tensor_mul(ms[:], m[:], s[:])
            tt = consts.tile((P, nch), F32)
            nc.vector.tensor_sub(tt[:], be_sb[:], ms[:])
            return s, tt

        s1, t1 = bn_params(ared1, 2, g1_sb, be1_sb)

        # ---- Phase B: normalize+relu y1, conv2, stats (512-wide) ----
        for c in range(NG):
            n0 = c * 512
            xn = []
            for o in range(2):
                x = xT_pool.tile((P, 512), BF16, name=f"xn{o}")
                nc.scalar.activation(
                    x[:],
                    y1h[o][:, n0 : n0 + 512],
                    AF.Relu,
                    bias=t1[:, o : o + 1],
                    scale=s1[:, o : o + 1],
                )
                xn.append(x)
            yps = psum_c.tile((P, 512), F32)
            for kc in range(2):
                nc.tensor.matmul(
                    yps[:],
                    lhsT=w2c[kc][:],
                    rhs=xn[kc][:],
                    start=(kc == 0),
                    stop=(kc == 1),
                )
            nc.scalar.activation(
                y2[:, n0 : n0 + 512],
                yps[:],
                AF.Identity,
                bias=b2_sb,
                accum_out=sums2[:, c : c + 1],
            )
            sc = scratch_pool.tile((P, 512), F32)
            nc.scalar.activation(
                sc[:], y2[:, n0 : n0 + 512], AF.Square, accum_out=sq2[:, c : c + 1]
            )

        # ---- BN2 stats AllReduce ----
        stats2 = consts.tile((P, 2), F32)
        nc.vector.reduce_sum(stats2[:, 0:1], sums2[:], axis=mybir.AxisListType.X)
        nc.vector.reduce_sum(stats2[:, 1:2], sq2[:], axis=mybir.AxisListType.X)
        st2_in = dram.tile((P, 2), F32)
        st2_out = dram.tile((P, 2), F32)
        nc.gpsimd.dma_start(st2_in[:], stats2[:])
        nc.gpsimd.collective_compute(
            "AllReduce",
            mybir.AluOpType.add,
            replica_groups=[list(range(N_CORES))],
            ins=[st2_in.opt()],
            outs=[st2_out.opt()],
        )
        ared2 = consts.tile((P, 2), F32)
        nc.gpsimd.dma_start(ared2[:], st2_out[:])

        s2, t2 = bn_params(ared2, 1, g2_sb, be2_sb)

        # ---- Phase C: normalize+relu y2 -> out (bf16) ----
        CW = 2048
        for c in range(N // CW):
            oc = outc_pool.tile((P, CW), BF16)
            nc.scalar.activation(
                oc[:],
                y2[:, c * CW : (c + 1) * CW],
                AF.Relu,
                bias=t2[:, 0:1],
                scale=s2[:, 0:1],
            )
            nc.sync.dma_start(out[:, c * CW : (c + 1) * CW], oc[:])

    import bass_rust

    # Walrus instruction structs hold a single sync wait; this pass splits
    # multi-wait instructions by inserting EventSemaphore (2-wait) preludes.
    bass_rust.generate_event_semaphores(nc)
    return nc


# ---------------- host-side runner ----------------

_lock = threading.Lock()
_st = {}


_mesh_lock = threading.Lock()


def _mesh_sharding():
    import jax
    from jax.sharding import Mesh, NamedSharding, PartitionSpec

    with _mesh_lock:
        if "sharding" not in _st:
            devices = jax.devices()[:N_CORES]
            mesh = Mesh(np.asarray(devices), ("core",))
            _st["mesh"] = mesh
            _st["sharding"] = NamedSharding(mesh, PartitionSpec("core"))
    return _st["sharding"]


def _compile_locked():
    if "compiled" in _st:
        return
    import jax
    from jax.experimental.shard_map import shard_map
    from jax.sharding import PartitionSpec

    from concourse import bass2jax

    try:
        jax.config.update("jax_compilation_cache_dir", "/tmp/jax_pc_cache")
        jax.config.update("jax_persistent_cache_min_compile_time_secs", 0.0)
        jax.config.update("jax_persistent_cache_min_entry_size_bytes", 0)
    except Exception:
        pass

    bass2jax.install_neuronx_cc_hook()
    _mesh_sharding()
    mesh = _st["mesh"]

    nc = _build_nc()

    partition_name = nc.partition_id_tensor.name if nc.partition_id_tensor else None
    in_names = []
    out_names = []
    out_avals = []
    for alloc in nc.m.functions[0].allocations:
        if not isinstance(alloc, mybir.MemoryLocationSet):
            continue
        name = alloc.memorylocations[0].name
        if alloc.kind == "ExternalInput":
            if name != partition_name:
                in_names.append(name)
        elif alloc.kind == "ExternalOutput":
            out_names.append(name)
            shape = tuple(alloc.tensor_shape)
            dtype = mybir.dt.np(alloc.dtype)
            out_avals.append(jax.core.ShapedArray(shape, dtype))
    n_params = len(in_names)
    n_outs = len(out_avals)
    all_names = list(in_names) + list(out_names)
    if partition_name is not None:
        all_names.append(partition_name)
    donate = tuple(range(n_params, n_params + n_outs))

    def _body(*args):
        operands = list(args)
        if partition_name is not None:
            operands.append(bass2jax.partition_id_tensor())
        outs = bass2jax._bass_exec_p.bind(
            *operands,
            out_avals=tuple(out_avals),
            in_names=tuple(all_names),
            out_names=tuple(out_names),
            lowering_input_output_aliases=(),
            sim_require_finite=True,
            sim_require_nnan=True,
            nc=nc,
        )
        return tuple(outs)

    in_specs = (PartitionSpec("core"),) * (n_params + n_outs)
    out_specs = (PartitionSpec("core"),) * n_outs
    sharded = jax.jit(
        shard_map(
            _body, mesh=mesh, in_specs=in_specs, out_specs=out_specs, check_rep=False
        ),
        donate_argnums=donate,
        keep_unused=True,
    )

    def g_aval(name):
        for alloc in nc.m.functions[0].allocations:
            if (
                isinstance(alloc, mybir.MemoryLocationSet)
                and alloc.memorylocations[0].name == name
            ):
                shape = tuple(alloc.tensor_shape)
                return jax.ShapeDtypeStruct(
                    (N_CORES * shape[0], *shape[1:]), mybir.dt.np(alloc.dtype)
                )
        raise KeyError(name)

    args = [g_aval(n) for n in in_names] + [g_aval(n) for n in out_names]
    compiled = sharded.lower(*args).compile()
    _st["in_names"] = in_names
    _st["out_names"] = out_names
    _st["out_shapes"] = [tuple(a.shape) for a in out_avals]
    _st["out_dtypes"] = [a.dtype for a in out_avals]

    # Donated output buffers are created on-device (a 16MB zeros upload
    # through the tunnel would cost ~0.4s otherwise).
    import jax.numpy as jnp

    zero_shapes = [
        ((N_CORES * s[0], *s[1:]), d)
        for s, d in zip(_st["out_shapes"], _st["out_dtypes"])
    ]
    _st["zeros_fn"] = jax.jit(
        lambda: tuple(jnp.zeros(s, d) for s, d in zero_shapes),
        out_shardings=_st["sharding"],
    ).lower().compile()
    # "compiled" marks the whole state as ready; commit it last.
    _st["compiled"] = compiled


def _ensure_compiled():
    with _lock:
        _compile_locked()


def _bg_compile():
    try:
        _ensure_compiled()
    except Exception:
        with _lock:
            _st.pop("compiled", None)


def _bg_mesh():
    # Axon backend init (~0.6s) runs concurrently with the cffi/bass init
    # the compile thread triggers; both native, so they overlap.
    try:
        _mesh_sharding()
    except Exception:
        pass


threading.Thread(target=_bg_mesh, daemon=True).start()
threading.Thread(target=_bg_compile, daemon=True).start()


def _prep_globals(inputs):
    """Build the concatenated (8*d0, ...) host arrays keyed by input name."""
    xyz1 = np.ascontiguousarray(inputs["xyz1"], dtype=np.float32)  # [B,3,N]
    xyz2 = np.ascontiguousarray(inputs["xyz2"], dtype=np.float32)  # [B,3,S]
    points1 = np.asarray(inputs["points1"], dtype=np.float32)  # [B,D1,N]
    points2 = np.asarray(inputs["points2"], dtype=np.float32)  # [B,D2,S]
    w1 = np.asarray(inputs["w1"], dtype=np.float32)
    w2 = np.asarray(inputs["w2"], dtype=np.float32)

    x1s = xyz1 * xyz1
    n1 = (x1s[:, 0] + x1s[:, 1]) + x1s[:, 2]  # [B,N], matches jnp sum order
    x2s = xyz2 * xyz2
    n2 = (x2s[:, 0] + x2s[:, 1]) + x2s[:, 2]  # [B,S]

    dl_full = np.empty((B, 5, N), np.float32)
    dl_full[:, 0:3] = 2.0 * xyz1
    dl_full[:, 3] = n1
    dl_full[:, 4] = -1.0
    v = dl_full.reshape(B, 5, NT, P)
    dlcat = np.zeros((B, 20, 2816), np.float32)
    dlcat[:, 0:5, :] = v[:, :, 0::3, :].reshape(B, 5, -1)
    dlcat[:, 5:10, : 21 * P] = v[:, :, 1::3, :].reshape(B, 5, -1)
    dlcat[:, 10:15, : 21 * P] = v[:, :, 2::3, :].reshape(B, 5, -1)
    dlcat[:, 15:18, :S] = xyz2
    dlcat[:, 18, :S] = -1.0
    dlcat[:, 19, :S] = n2

    wcat = np.zeros((Cin + C1, C1), BF16_NP)
    wcat[:Cin] = np.ascontiguousarray(w1.T).astype(BF16_NP)
    wcat[Cin:, :C2] = np.ascontiguousarray(w2.T).astype(BF16_NP)

    bncat = np.empty((P, 9), np.float32)
    bncat[:, 0:2] = np.asarray(inputs["b1"], np.float32).reshape(2, P).T
    bncat[:, 2:4] = np.asarray(inputs["gamma1"], np.float32).reshape(2, P).T
    bncat[:, 4:6] = np.asarray(inputs["beta1"], np.float32).reshape(2, P).T
    bncat[:, 6] = np.asarray(inputs["b2"], np.float32)
    bncat[:, 7] = np.asarray(inputs["gamma2"], np.float32)
    bncat[:, 8] = np.asarray(inputs["beta2"], np.float32)

    g = {
        "points1": points1.reshape(B * D1, N).astype(BF16_NP),
        "p2t": np.ascontiguousarray(points2.transpose(0, 2, 1))
        .reshape(B * S, D2)
        .astype(BF16_NP),
        "dlcat": dlcat.reshape(B * 20, 2816),
        "wcat": np.tile(wcat, (B, 1)),
        "bncat": np.tile(bncat, (B, 1)),
    }
    return g


def kernel(**inputs):
    import jax

    timing = os.environ.get("KERNEL_TIMING", "0") == "1"
    tlog = (lambda *a: print(*a, file=sys.stderr, flush=True)) if timing else (lambda *a: None)

    t0 = time.time()
    g = _prep_globals(inputs)
    sh = _mesh_sharding()
    tlog(f"[k] prep {time.time()-t0:.3f}s")

    # Async h2d from a helper thread (device_put dispatch can block while a
    # prior transfer drains); transfers overlap the build+compile below.
    t0 = time.time()
    dev = {}
    order = ["points1", "p2t", "dlcat", "wcat", "bncat"]

    def _h2d():
        for name in order:
            dev[name] = jax.device_put(g[name], sh)
        for v in dev.values():
            jax.block_until_ready(v)

    h2d_thread = threading.Thread(target=_h2d, daemon=True)
    h2d_thread.start()
    tlog(f"[k] h2d dispatch {time.time()-t0:.3f}s")

    t0 = time.time()
    _ensure_compiled()
    tlog(f"[k] compile-join {time.time()-t0:.3f}s")

    t0 = time.time()
    dev_zeros = _st["zeros_fn"]()
    h2d_thread.join()
    tlog(f"[k] h2d join {time.time()-t0:.3f}s")

    t0 = time.time()
    compiled = _st["compiled"]
    args = [dev[n] for n in _st["in_names"]] + list(dev_zeros)
    out_arrs = compiled(*args)
    jax.block_until_ready(out_arrs)
    tlog(f"[k] execute {time.time()-t0:.3f}s")

    t0 = time.time()
    outg = np.asarray(out_arrs[0])  # (8*C2, N) bf16
    res = outg.reshape(N_CORES, C2, N).astype(np.float32)
    tlog(f"[k] fetch+convert {time.time()-t0:.3f}s")
    last["exec_time_ns"] = None
    return res
